# revision 1
# baseline (speedup 1.0000x reference)
"""Trainium2 Bass kernel for stacked-Linear dense MLP:
    out[1024, 32768] = x[1024, 512] @ W[32768, 512].T + b[32768]

Strategy: column-parallel over 8 NeuronCores. Core c owns W rows
[c*4096, (c+1)*4096) -> output columns of the same range. x replicated.
On-chip: bf16 matmul (fp32 PSUM accumulate), bias added on DVE during
PSUM->SBUF evacuation, fp32 output DMA'd out.
"""

import sys

sys.path.insert(0, "/opt/trn_rl_repo")

import numpy as np
import ml_dtypes

# ---- problem constants (hardcoded per contract) ----
B = 1024          # batch (matmul M)
K = 512           # hidden size (contraction)
N_TOTAL = 32768   # hidden_size * map_element_size
N_CORES = 8
NS = N_TOTAL // N_CORES  # 4096 output cols per core

KT = K // 128     # 4 k-tiles
MT = B // 128     # 8 m-tiles
NCH = NS // 512   # 8 n-chunks of 512 (one PSUM bank each)

_CACHE = {}


def _build_program():
    import concourse.bacc as bacc
    import concourse.mybir as mybir
    from concourse.bass import ts
    from concourse.tile import TileContext
    from contextlib import ExitStack

    nc = bacc.Bacc("TRN2", target_bir_lowering=False, debug=False, num_devices=N_CORES)

    xT = nc.dram_tensor("xT", [K, B], mybir.dt.bfloat16, kind="ExternalInput").ap()
    wT = nc.dram_tensor("wT", [K, NS], mybir.dt.bfloat16, kind="ExternalInput").ap()
    bias = nc.dram_tensor("bias", [1, NS], mybir.dt.float32, kind="ExternalInput").ap()
    out = nc.dram_tensor("out", [B, NS], mybir.dt.float32, kind="ExternalOutput").ap()

    with TileContext(nc) as tc:
        with ExitStack() as ctx:
            const = ctx.enter_context(tc.tile_pool(name="const", bufs=1))
            outp = ctx.enter_context(tc.tile_pool(name="outp", bufs=4))
            psum = ctx.enter_context(tc.tile_pool(name="psum", bufs=8, space="PSUM"))

            xT_r = xT.rearrange("(kt p) m -> p kt m", p=128)  # [128, KT, B]
            wT_r = wT.rearrange("(kt p) n -> p kt n", p=128)  # [128, KT, NS]

            xt_tiles = []
            for m in range(MT):
                t = const.tile([128, KT, 128], mybir.dt.bfloat16, tag=f"xt{m}")
                nc.sync.dma_start(t[:], xT_r[:, :, ts(m, 128)])
                xt_tiles.append(t)

            wt_tiles = []
            for n in range(NCH):
                t = const.tile([128, KT, 512], mybir.dt.bfloat16, tag=f"wt{n}")
                nc.sync.dma_start(t[:], wT_r[:, :, ts(n, 512)])
                wt_tiles.append(t)

            bias_sb = const.tile([128, NS], mybir.dt.float32, tag="bias")
            nc.sync.dma_start(bias_sb[0:1, :], bias)
            nc.gpsimd.partition_broadcast(bias_sb[:], bias_sb[0:1, :])

            for m in range(MT):
                for n in range(NCH):
                    ps = psum.tile([128, 512], mybir.dt.float32)
                    for k in range(KT):
                        nc.tensor.matmul(
                            ps[:],
                            lhsT=xt_tiles[m][:, k, :],
                            rhs=wt_tiles[n][:, k, :],
                            start=(k == 0),
                            stop=(k == KT - 1),
                        )
                    ot = outp.tile([128, 512], mybir.dt.float32)
                    nc.vector.tensor_add(ot[:], ps[:], bias_sb[:, ts(n, 512)])
                    nc.sync.dma_start(out[ts(m, 128), ts(n, 512)], ot[:])

    nc.compile()
    return nc


def _get_program():
    if "nc" not in _CACHE:
        _CACHE["nc"] = _build_program()
    return _CACHE["nc"]


def _prep_inputs(x, W, b):
    bf16 = ml_dtypes.bfloat16
    xT = np.ascontiguousarray(x.T).astype(bf16)  # [K, B]
    in_maps = []
    for c in range(N_CORES):
        sl = slice(c * NS, (c + 1) * NS)
        wTc = np.ascontiguousarray(W[sl, :].T).astype(bf16)  # [K, NS]
        bc = np.ascontiguousarray(b[sl].astype(np.float32).reshape(1, NS))
        in_maps.append({"xT": xT, "wT": wTc, "bias": bc})
    return in_maps


def _run(x, W, b, trace=False):
    from concourse.bass_utils import run_bass_kernel_spmd

    nc = _get_program()
    in_maps = _prep_inputs(x, W, b)
    res = run_bass_kernel_spmd(nc, in_maps, list(range(N_CORES)), trace=trace)
    _CACHE["last_result"] = res
    out = np.concatenate([r["out"] for r in res.results], axis=1)
    return out.astype(np.float32)


def kernel(x, W, b):
    return _run(x, W, b, trace=False)


def kernel_profiled(x, W, b):
    """Same as kernel() but with NTFF tracing; returns (out, BassKernelResults)."""
    out = _run(x, W, b, trace=True)
    return out, _CACHE["last_result"]


# revision 4
# speedup vs baseline: 1.1339x; 1.1339x over previous
"""Trainium2 Bass kernel for stacked-Linear dense MLP:
    out[1024, 32768] = x[1024, 512] @ W[32768, 512].T + b[32768]

Strategy: column-parallel over 8 NeuronCores. Core c owns W rows
[c*4096, (c+1)*4096) -> output columns of the same range; x replicated.
On-chip: bf16 matmul (fp32 PSUM accumulate), bias added on DVE during
PSUM->SBUF evacuation, fp32 output DMA'd out.

Perf structure:
  - Host pre-arranges x/W into SBUF-image layouts so every DMA moves
    4-8 KiB contiguous per partition (max descriptor efficiency).
  - bias DMA + gpsimd partition_broadcast issued first (hidden under W loads).
  - Input DMAs split across the two HWDGE rings (scalar: bias+x, sync: W).
  - 8 PE warmup matmuls on a scratch tile un-throttle the HAM clock gate
    before the first real matmul.
  - Output staged in [128, 2048] fp32 tiles, DMA'd alternately on the
    sync/scalar rings.
"""

import sys

sys.path.insert(0, "/opt/trn_rl_repo")

import numpy as np
import ml_dtypes

# ---- problem constants (hardcoded per contract) ----
B = 1024          # batch (matmul M)
K = 512           # hidden size (contraction)
N_TOTAL = 32768   # hidden_size * map_element_size
N_CORES = 8
NS = N_TOTAL // N_CORES  # 4096 output cols per core

KT = K // 128     # 4 k-tiles
MT = B // 128     # 8 m-tiles
NCH = NS // 512   # 8 n-chunks of 512 (one PSUM bank each)

_CACHE = {}


def _build_program():
    import concourse.bacc as bacc
    import concourse.mybir as mybir
    from concourse.bass import ds, ts
    from concourse.tile import TileContext
    from contextlib import ExitStack

    nc = bacc.Bacc("TRN2", target_bir_lowering=False, debug=False, num_devices=N_CORES)

    # host-prepared SBUF-image layouts (see _prep_inputs)
    xh = nc.dram_tensor("xh", [128, KT, B], mybir.dt.bfloat16, kind="ExternalInput").ap()
    wh = nc.dram_tensor("wh", [128, NCH, KT, 512], mybir.dt.bfloat16, kind="ExternalInput").ap()
    bias = nc.dram_tensor("bias", [1, NS], mybir.dt.float32, kind="ExternalInput").ap()
    out = nc.dram_tensor("out", [B, NS], mybir.dt.float32, kind="ExternalOutput").ap()

    with TileContext(nc) as tc:
        with ExitStack() as ctx:
            const = ctx.enter_context(tc.tile_pool(name="const", bufs=1))
            outp = ctx.enter_context(tc.tile_pool(name="outp", bufs=4))
            psum = ctx.enter_context(tc.tile_pool(name="psum", bufs=7, space="PSUM"))
            wpool = ctx.enter_context(tc.tile_pool(name="wpool", bufs=1))

            # --- bias first: tiny DMA + gpsimd broadcast, hidden under W loads
            bias_sb = const.tile([128, NS], mybir.dt.float32, tag="bias")
            nc.scalar.dma_start(bias_sb[0:1, :], bias)
            nc.gpsimd.partition_broadcast(bias_sb[:], bias_sb[0:1, :])

            # --- PE warmup: un-throttle HAM while input DMAs run
            warm = const.tile([128, 512], mybir.dt.bfloat16, tag="warm")
            warm_ps = psum.tile([128, 512], mybir.dt.float32, tag="warmps", bufs=1)
            nc.vector.memset(warm[:], 0)
            for _ in range(8):
                nc.tensor.matmul(
                    warm_ps[:], lhsT=warm[:, 0:128], rhs=warm[:], start=True, stop=True
                )
            warm_sink = const.tile([128, 512], mybir.dt.float32, tag="warmsink")
            nc.scalar.copy(warm_sink[:], warm_ps[:])  # keep warmups live (no DCE)

            # --- x: one DMA on the scalar ring (parallel with W on sync ring)
            xh_sb = const.tile([128, KT, B], mybir.dt.bfloat16, tag="xh")
            nc.scalar.dma_start(xh_sb[:], xh)

            # --- W: 8 chunks on the sync ring, in consumption order
            wt_tiles = []
            for n in range(NCH):
                t = wpool.tile([128, KT, 512], mybir.dt.bfloat16, tag=f"wt{n}")
                nc.sync.dma_start(t[:], wh[:, n])
                wt_tiles.append(t)

            # --- main loop: m-rows outer, PE stays dense
            for m in range(MT):
                for half in range(2):
                    ot = outp.tile([128, 4 * 512], mybir.dt.float32)
                    for i in range(4):
                        n = half * 4 + i
                        ps = psum.tile([128, 512], mybir.dt.float32)
                        for k in range(KT):
                            nc.tensor.matmul(
                                ps[:],
                                lhsT=xh_sb[:, k, ds(m * 128, 128)],
                                rhs=wt_tiles[n][:, k, :],
                                start=(k == 0),
                                stop=(k == KT - 1),
                            )
                        nc.vector.tensor_add(
                            ot[:, ds(i * 512, 512)], ps[:], bias_sb[:, ds(n * 512, 512)]
                        )
                    eng = nc.sync if half == 0 else nc.scalar
                    eng.dma_start(out[ts(m, 128), ds(half * 2048, 2048)], ot[:])

    nc.compile()
    return nc


def _get_program():
    if "nc" not in _CACHE:
        _CACHE["nc"] = _build_program()
    return _CACHE["nc"]


def _prep_inputs(x, W, b):
    bf16 = ml_dtypes.bfloat16
    x = np.asarray(x, dtype=np.float32)
    W = np.asarray(W, dtype=np.float32)
    b = np.asarray(b, dtype=np.float32)
    # xh[p, kt, m] = x[m, kt*128 + p]
    xh = np.ascontiguousarray(
        x.T.reshape(KT, 128, B).transpose(1, 0, 2)
    ).astype(bf16)
    in_maps = []
    for c in range(N_CORES):
        sl = slice(c * NS, (c + 1) * NS)
        # wh[p, n, kt, j] = W[c*NS + n*512 + j, kt*128 + p]
        wh = np.ascontiguousarray(
            W[sl, :].T.reshape(KT, 128, NCH, 512).transpose(1, 2, 0, 3)
        ).astype(bf16)
        bc = np.ascontiguousarray(b[sl].reshape(1, NS))
        in_maps.append({"xh": xh, "wh": wh, "bias": bc})
    return in_maps


def _run(x, W, b, trace=False):
    from concourse.bass_utils import run_bass_kernel_spmd

    nc = _get_program()
    in_maps = _prep_inputs(x, W, b)
    res = run_bass_kernel_spmd(nc, in_maps, list(range(N_CORES)), trace=trace)
    _CACHE["last_result"] = res
    out = np.concatenate([r["out"] for r in res.results], axis=1)
    return out.astype(np.float32)


def kernel(x, W, b):
    return _run(x, W, b, trace=False)


def kernel_profiled(x, W, b):
    """Same as kernel() but with NTFF tracing; returns (out, BassKernelResults)."""
    out = _run(x, W, b, trace=True)
    return out, _CACHE["last_result"]


# revision 5
# speedup vs baseline: 1.2034x; 1.0612x over previous
"""Trainium2 Bass kernel for stacked-Linear dense MLP:
    out[1024, 32768] = x[1024, 512] @ W[32768, 512].T + b[32768]

Strategy: column-parallel over 8 NeuronCores. Core c owns W rows
[c*4096, (c+1)*4096) -> output columns of the same range; x replicated.
On-chip: bf16 matmul (fp32 PSUM accumulate), bias added on DVE during
PSUM->SBUF evacuation, fp32 output DMA'd out.

Perf structure:
  - Host pre-arranges x/W into SBUF-image layouts so every DMA moves
    2-8 KiB contiguous per partition (max descriptor efficiency).
  - n-OUTER loop: each W chunk (512 KiB, ~1.5us load) feeds 8 m-tile
    matmul groups (~6.8us of PE work), so the PE runs dense from the
    moment the first chunk lands instead of waiting for all of W
    (concurrent DMAs on a ring share bandwidth and complete together).
  - bias DMA + gpsimd partition_broadcast issued first (hidden under loads).
  - Input DMAs split across the two HWDGE rings (scalar: bias+x, sync: W).
  - PE warmup matmuls on a scratch tile un-throttle the HAM clock gate.
  - Output DMAs alternate between the sync/scalar rings.
"""

import sys

sys.path.insert(0, "/opt/trn_rl_repo")

import numpy as np
import ml_dtypes

# ---- problem constants (hardcoded per contract) ----
B = 1024          # batch (matmul M)
K = 512           # hidden size (contraction)
N_TOTAL = 32768   # hidden_size * map_element_size
N_CORES = 8
NS = N_TOTAL // N_CORES  # 4096 output cols per core

KT = K // 128     # 4 k-tiles
MT = B // 128     # 8 m-tiles
NCH = NS // 512   # 8 n-chunks of 512 (one PSUM bank each)

_CACHE = {}


def _build_program():
    import concourse.bacc as bacc
    import concourse.mybir as mybir
    from concourse.bass import ds, ts
    from concourse.tile import TileContext
    from contextlib import ExitStack

    nc = bacc.Bacc("TRN2", target_bir_lowering=False, debug=False, num_devices=N_CORES)

    # host-prepared SBUF-image layouts (see _prep_inputs)
    xh = nc.dram_tensor("xh", [128, KT, B], mybir.dt.bfloat16, kind="ExternalInput").ap()
    wh = nc.dram_tensor("wh", [128, NCH, KT, 512], mybir.dt.bfloat16, kind="ExternalInput").ap()
    bias = nc.dram_tensor("bias", [1, NS], mybir.dt.float32, kind="ExternalInput").ap()
    out = nc.dram_tensor("out", [B, NS], mybir.dt.float32, kind="ExternalOutput").ap()

    with TileContext(nc) as tc:
        with ExitStack() as ctx:
            const = ctx.enter_context(tc.tile_pool(name="const", bufs=1))
            outp = ctx.enter_context(tc.tile_pool(name="outp", bufs=6))
            psum = ctx.enter_context(tc.tile_pool(name="psum", bufs=7, space="PSUM"))
            wpool = ctx.enter_context(tc.tile_pool(name="wpool", bufs=1))

            # --- bias first: tiny DMA + gpsimd broadcast, hidden under loads
            bias_sb = const.tile([128, NS], mybir.dt.float32, tag="bias")
            nc.scalar.dma_start(bias_sb[0:1, :], bias)
            nc.gpsimd.partition_broadcast(bias_sb[:], bias_sb[0:1, :])

            # --- PE warmup: un-throttle HAM while input DMAs run
            warm = const.tile([128, 512], mybir.dt.bfloat16, tag="warm")
            warm_ps = psum.tile([128, 512], mybir.dt.float32, tag="warmps", bufs=1)
            nc.vector.memset(warm[:], 0)
            for _ in range(10):
                nc.tensor.matmul(
                    warm_ps[:], lhsT=warm[:, 0:128], rhs=warm[:], start=True, stop=True
                )
            warm_sink = const.tile([128, 512], mybir.dt.float32, tag="warmsink")
            nc.vector.tensor_copy(warm_sink[:], warm_ps[:])  # keep warmups live

            # --- x: one DMA on the scalar ring (parallel with W on sync ring)
            xh_sb = const.tile([128, KT, B], mybir.dt.bfloat16, tag="xh")
            nc.scalar.dma_start(xh_sb[:], xh)

            # --- W chunks on the sync ring, in consumption order
            wt_tiles = []
            for n in range(NCH):
                t = wpool.tile([128, KT, 512], mybir.dt.bfloat16, tag=f"wt{n}")
                nc.sync.dma_start(t[:], wh[:, n])
                wt_tiles.append(t)

            # --- main loop: n-chunks outer so PE tracks W arrival
            for n in range(NCH):
                for m in range(MT):
                    ps = psum.tile([128, 512], mybir.dt.float32)
                    for k in range(KT):
                        nc.tensor.matmul(
                            ps[:],
                            lhsT=xh_sb[:, k, ds(m * 128, 128)],
                            rhs=wt_tiles[n][:, k, :],
                            start=(k == 0),
                            stop=(k == KT - 1),
                        )
                    ot = outp.tile([128, 512], mybir.dt.float32)
                    nc.vector.tensor_add(ot[:], ps[:], bias_sb[:, ds(n * 512, 512)])
                    eng = nc.sync if (n * MT + m) % 2 == 0 else nc.scalar
                    eng.dma_start(out[ts(m, 128), ds(n * 512, 512)], ot[:])

    nc.compile()
    return nc


def _get_program():
    if "nc" not in _CACHE:
        _CACHE["nc"] = _build_program()
    return _CACHE["nc"]


def _prep_inputs(x, W, b):
    bf16 = ml_dtypes.bfloat16
    x = np.asarray(x, dtype=np.float32)
    W = np.asarray(W, dtype=np.float32)
    b = np.asarray(b, dtype=np.float32)
    # xh[p, kt, m] = x[m, kt*128 + p]
    xh = np.ascontiguousarray(
        x.T.reshape(KT, 128, B).transpose(1, 0, 2)
    ).astype(bf16)
    in_maps = []
    for c in range(N_CORES):
        sl = slice(c * NS, (c + 1) * NS)
        # wh[p, n, kt, j] = W[c*NS + n*512 + j, kt*128 + p]
        wh = np.ascontiguousarray(
            W[sl, :].T.reshape(KT, 128, NCH, 512).transpose(1, 2, 0, 3)
        ).astype(bf16)
        bc = np.ascontiguousarray(b[sl].reshape(1, NS))
        in_maps.append({"xh": xh, "wh": wh, "bias": bc})
    return in_maps


def _run(x, W, b, trace=False):
    from concourse.bass_utils import run_bass_kernel_spmd

    nc = _get_program()
    in_maps = _prep_inputs(x, W, b)
    res = run_bass_kernel_spmd(nc, in_maps, list(range(N_CORES)), trace=trace)
    _CACHE["last_result"] = res
    out = np.concatenate([r["out"] for r in res.results], axis=1)
    return out.astype(np.float32)


def kernel(x, W, b):
    return _run(x, W, b, trace=False)


def kernel_profiled(x, W, b):
    """Same as kernel() but with NTFF tracing; returns (out, BassKernelResults)."""
    out = _run(x, W, b, trace=True)
    return out, _CACHE["last_result"]
